# revision 17
# baseline (speedup 1.0000x reference)
"""Block-diagonal linear (BlockLinear) Trainium2 Bass kernel.

Problem: out[b, n, o] = sum_i x[b, n, i] * W[n, o, i] + bias[n, o]
  x: [1024, 1024, 64] f32, W: [1024, 64, 64] f32, bias: [1024, 64] f32

Sharding: block-parallel over n (num_blocks) across 8 NeuronCores;
each core owns 128 blocks. No inter-core communication.

Per-core algorithm (all fp32):
  - The contraction dim i is innermost in DRAM, so x tiles arrive in
    SBUF as [b=128 partitions, i free]. The tensor engine contracts over
    the partition dim, so x is transposed on chip: a PE transpose
    (x_tile.T @ I) over a [128b, 128] tile covering TWO blocks
    (2 x 64 = 128) yields xT [i2=128, b=128] in PSUM at full array width.
  - Weights are expanded on chip into block-pair block-diagonal tiles
    W2[pair] = [[W[2p].T, 0], [0, W[2p+1].T]]  (shape [128, 128]),
    so a single fp32 matmul  xT.T @ W2  = [b=128, o2=128] computes two
    blocks at once with K=128 (full partition utilization). Only the
    compact 2MB W.T is DMA'd; zeros + layout are built by DVE.
  - Bias is DMA'd compact (32KB), partition-broadcast on chip by GPSIMD,
    and added by the DVE during the PSUM->SBUF copy of the output.
  - All DRAM<->SBUF DMAs move >=2KB contiguous per partition (line rate).

The kernel is memory-bound: per core it streams 32MB x in + 32MB out
out at the measured ~300GB/s/core mixed R/W rate (~220us); PE/ACT/DVE
work hides underneath.
"""

import contextlib

import numpy as np

import concourse.bass as bass
import concourse.bacc as bacc
import concourse.tile as tile
from concourse import mybir
from concourse.bass_utils import run_bass_kernel_spmd

F32 = mybir.dt.float32

B = 1024          # batch
NB = 1024         # num_blocks (total)
DIN = 64
DOUT = 64
NCORES = 8
NB_C = NB // NCORES          # 128 blocks per core
CHUNK = 128                  # batch rows per tile (SBUF partitions)
NCHUNK = B // CHUNK          # 8
XH = 64                      # blocks per x DMA (16KB/partition)
OB = 32                      # blocks per out DMA (8KB/partition)
GRP = 8                      # blocks per PSUM bank group


def build_program(n_reps=1, xh=XH, ob=OB, pt_bufs=4, po_bufs=2,
                  xt_bufs=8, x_bufs=3, o_bufs=3, plain_mm_transpose=False,
                  out_engine="scalar"):
    """n_reps>1 wraps the main loop in a HW loop repeating the whole
    computation — used only for timing (amortizes dispatch overhead)."""
    nc = bacc.Bacc(
        "TRN2", target_bir_lowering=False, debug=False, num_devices=NCORES
    )
    x_d = nc.dram_tensor("x", [B, NB_C, DIN], F32, kind="ExternalInput")
    # compact stacked W.T: rows 0:64 = W[2p].T, rows 64:128 = W[2p+1].T
    w2c_d = nc.dram_tensor("w2c", [128, NB_C // 2, DOUT], F32,
                           kind="ExternalInput")
    bc_d = nc.dram_tensor("bc", [1, NB_C * DOUT], F32, kind="ExternalInput")
    id_d = nc.dram_tensor("ident", [128, 128], F32, kind="ExternalInput")
    o_d = nc.dram_tensor("out", [B, NB_C, DOUT], F32, kind="ExternalOutput")

    xa, w2ca, bca, ida, oa = (t.ap() for t in (x_d, w2c_d, bc_d, id_d, o_d))

    with tile.TileContext(nc) as tc:
        with (
            tc.tile_pool(name="const", bufs=1) as cpool,
            tc.tile_pool(name="xin", bufs=x_bufs) as xpool,
            tc.tile_pool(name="xt", bufs=xt_bufs) as xtpool,
            tc.tile_pool(name="pt", bufs=pt_bufs, space="PSUM") as ptpool,
            tc.tile_pool(name="po", bufs=po_bufs, space="PSUM") as popool,
            tc.tile_pool(name="oo", bufs=o_bufs) as opool,
        ):
            ident = cpool.tile([128, 128], F32)
            nc.sync.dma_start(ident[:], ida[:])

            # Constants ride the scalar HWDGE ring so the sync ring's FIFO
            # leads with the first x tiles (compute starts sooner).
            # --- on-chip W2 block-diagonal expansion (saves 2MB DMA) ---
            w2 = cpool.tile([128, NB_C // 2, 128], F32)
            w2c = xpool.tile([128, NB_C // 2, DOUT], F32, tag="x_t")  # borrow slot
            nc.scalar.dma_start(w2c[:], w2ca[:])
            nc.gpsimd.memset(w2[:], 0.0)
            nc.vector.tensor_copy(w2[0:64, :, 0:64], w2c[0:64, :, :])
            nc.vector.tensor_copy(w2[64:128, :, 64:128], w2c[64:128, :, :])

            # --- on-chip bias broadcast (saves 4MB DMA) ---
            bias_c = cpool.tile([1, NB_C * DOUT], F32)
            nc.scalar.dma_start(bias_c[:], bca[:])
            bb = cpool.tile([128, NB_C // GRP, GRP, DOUT], F32)
            nc.gpsimd.partition_broadcast(bb[:], bias_c[:])

            rep_cm = (
                tc.For_i(0, n_reps, 1) if n_reps > 1 else contextlib.nullcontext()
            )
            with rep_cm:
                main_body(nc, tc, xa, oa, w2, bb, ident,
                          xpool, xtpool, ptpool, popool, opool,
                          xh=xh, ob_sz=ob, plain_mm_transpose=plain_mm_transpose,
                          out_engine=out_engine)

    nc.compile()
    return nc


def main_body(nc, tc, xa, oa, w2, bb, ident,
              xpool, xtpool, ptpool, popool, opool,
              xh=XH, ob_sz=OB, plain_mm_transpose=False, out_engine="sync"):
    wr = getattr(nc, out_engine)
    for c in range(NCHUNK):
        for h in range(NB_C // xh):
            x_t = xpool.tile([CHUNK, xh, DIN], F32, tag="x_t")
            nc.sync.dma_start(
                x_t[:],
                xa[c * CHUNK:(c + 1) * CHUNK, h * xh:(h + 1) * xh, :],
            )
            for ob in range(xh // ob_sz):
                o_t = opool.tile([CHUNK, ob_sz, DOUT], F32)
                for gi in range(ob_sz // GRP):
                    blk0 = h * xh + ob * ob_sz + gi * GRP
                    g = blk0 // GRP
                    po = popool.tile([CHUNK, GRP, DOUT], F32)
                    for q in range(GRP // 2):
                        pair = blk0 // 2 + q
                        xoff = ob * ob_sz + gi * GRP + 2 * q
                        pt = ptpool.tile([128, CHUNK], F32)
                        if plain_mm_transpose:
                            nc.tensor.matmul(
                                pt[:], x_t[:, xoff:xoff + 2, :], ident[:],
                                start=True, stop=True,
                            )
                        else:
                            nc.tensor.transpose(
                                pt[:], x_t[:, xoff:xoff + 2, :], ident[:]
                            )
                        xts = xtpool.tile([128, CHUNK], F32)
                        nc.scalar.mul(xts[:], pt[:], 1.0)
                        nc.tensor.matmul(
                            po[:, 2 * q:2 * q + 2, :],
                            xts[:],
                            w2[:, pair, :],
                            start=True,
                            stop=True,
                        )
                    nc.vector.tensor_add(
                        o_t[:, gi * GRP:(gi + 1) * GRP, :],
                        po[:],
                        bb[:, g, :, :],
                    )
                nb0 = h * xh + ob * ob_sz
                wr.dma_start(
                    oa[c * CHUNK:(c + 1) * CHUNK, nb0:nb0 + ob_sz, :],
                    o_t[:],
                )


_PROGRAMS = {}


def get_program(n_reps=1):
    if n_reps not in _PROGRAMS:
        _PROGRAMS[n_reps] = build_program(n_reps)
    return _PROGRAMS[n_reps]


def prep_core_inputs(x, W, b, core):
    """Host-side shard + layout prep for one core."""
    n0, n1 = core * NB_C, (core + 1) * NB_C
    xs = np.ascontiguousarray(x[:, n0:n1, :], dtype=np.float32)
    Wk = W[n0:n1]                                  # [128, 64, 64] (n, o, i)
    WT = Wk.transpose(0, 2, 1)                     # [128, 64, 64] (n, i, o)
    # compact stacked layout [i2=128, pair, o]: rows 0:64 even blocks,
    # rows 64:128 odd blocks
    w2c = np.empty((128, NB_C // 2, DOUT), dtype=np.float32)
    w2c[:64] = WT[0::2].transpose(1, 0, 2)
    w2c[64:] = WT[1::2].transpose(1, 0, 2)
    bc = np.ascontiguousarray(b[n0:n1].reshape(1, NB_C * DOUT),
                              dtype=np.float32)
    ident = np.eye(128, dtype=np.float32)
    return {"x": xs, "w2c": w2c, "bc": bc, "ident": ident}


def make_in_maps(x, W, b):
    return [prep_core_inputs(x, W, b, k) for k in range(NCORES)]


def kernel(x, W, b):
    nc = get_program()
    in_maps = make_in_maps(x, W, b)
    res = run_bass_kernel_spmd(nc, in_maps, list(range(NCORES)))
    out = np.concatenate([res.results[k]["out"] for k in range(NCORES)], axis=1)
    return out


# revision 21
# speedup vs baseline: 1.0227x; 1.0227x over previous
"""Block-diagonal linear (BlockLinear) Trainium2 Bass kernel.

Problem: out[b, n, o] = sum_i x[b, n, i] * W[n, o, i] + bias[n, o]
  x: [1024, 1024, 64] f32, W: [1024, 64, 64] f32, bias: [1024, 64] f32

Sharding: block-parallel over n (num_blocks) across 8 NeuronCores;
each core owns 128 blocks. No inter-core communication.

Per-core algorithm (all fp32):
  - The contraction dim i is innermost in DRAM, so x tiles arrive in
    SBUF as [b=128 partitions, i free]. The tensor engine contracts over
    the partition dim, so x is transposed on chip: a PE transpose
    (x_tile.T @ I) over a [128b, 128] tile covering TWO blocks
    (2 x 64 = 128) yields xT [i2=128, b=128] in PSUM at full array width.
  - Weights are expanded on chip into block-pair block-diagonal tiles
    W2[pair] = [[W[2p].T, 0], [0, W[2p+1].T]]  (shape [128, 128]),
    so a single fp32 matmul  xT.T @ W2  = [b=128, o2=128] computes two
    blocks at once with K=128 (full partition utilization). Only the
    compact 2MB W.T is DMA'd; zeros + layout are built by DVE.
  - Bias is DMA'd compact (32KB), partition-broadcast on chip by GPSIMD,
    and added by the DVE during the PSUM->SBUF copy of the output.
  - All DRAM<->SBUF DMAs move >=2KB contiguous per partition (line rate).

The kernel is memory-bound: per core it streams 32MB x in + 32MB out
out at the measured ~300GB/s/core mixed R/W rate (~220us); PE/ACT/DVE
work hides underneath.
"""

import contextlib

import numpy as np

import concourse.bass as bass
import concourse.bacc as bacc
import concourse.tile as tile
from concourse import mybir
from concourse.bass_utils import run_bass_kernel_spmd

F32 = mybir.dt.float32

B = 1024          # batch
NB = 1024         # num_blocks (total)
DIN = 64
DOUT = 64
NCORES = 8
NB_C = NB // NCORES          # 128 blocks per core
CHUNK = 128                  # batch rows per tile (SBUF partitions)
NCHUNK = B // CHUNK          # 8
XH = 64                      # blocks per x DMA (16KB/partition)
OB = 32                      # blocks per out DMA (8KB/partition)
GRP = 8                      # blocks per PSUM bank group


def build_program(n_reps=1, xh=XH, ob=OB, pt_bufs=4, po_bufs=2,
                  xt_bufs=8, x_bufs=3, o_bufs=3, plain_mm_transpose=False,
                  out_engine="scalar", split_first=0):
    """n_reps>1 wraps the main loop in a HW loop repeating the whole
    computation — used only for timing (amortizes dispatch overhead)."""
    nc = bacc.Bacc(
        "TRN2", target_bir_lowering=False, debug=False, num_devices=NCORES
    )
    x_d = nc.dram_tensor("x", [B, NB_C, DIN], F32, kind="ExternalInput")
    # compact stacked W.T: rows 0:64 = W[2p].T, rows 64:128 = W[2p+1].T
    w2c_d = nc.dram_tensor("w2c", [128, NB_C // 2, DOUT], F32,
                           kind="ExternalInput")
    bc_d = nc.dram_tensor("bc", [1, NB_C * DOUT], F32, kind="ExternalInput")
    id_d = nc.dram_tensor("ident", [128, 128], F32, kind="ExternalInput")
    o_d = nc.dram_tensor("out", [B, NB_C, DOUT], F32, kind="ExternalOutput")

    xa, w2ca, bca, ida, oa = (t.ap() for t in (x_d, w2c_d, bc_d, id_d, o_d))

    with tile.TileContext(nc) as tc:
        with (
            tc.tile_pool(name="const", bufs=1) as cpool,
            tc.tile_pool(name="xin", bufs=x_bufs) as xpool,
            tc.tile_pool(name="xt", bufs=xt_bufs) as xtpool,
            tc.tile_pool(name="pt", bufs=pt_bufs, space="PSUM") as ptpool,
            tc.tile_pool(name="po", bufs=po_bufs, space="PSUM") as popool,
            tc.tile_pool(name="oo", bufs=o_bufs) as opool,
        ):
            ident = cpool.tile([128, 128], F32)
            nc.sync.dma_start(ident[:], ida[:])

            # Constants ride the scalar HWDGE ring so the sync ring's FIFO
            # leads with the first x tiles (compute starts sooner).
            # --- on-chip W2 block-diagonal expansion (saves 2MB DMA) ---
            w2 = cpool.tile([128, NB_C // 2, 128], F32)
            w2c = xpool.tile([128, NB_C // 2, DOUT], F32, tag="x_t")  # borrow slot
            nc.scalar.dma_start(w2c[:], w2ca[:])
            nc.gpsimd.memset(w2[:], 0.0)
            nc.vector.tensor_copy(w2[0:64, :, 0:64], w2c[0:64, :, :])
            nc.vector.tensor_copy(w2[64:128, :, 64:128], w2c[64:128, :, :])

            # --- on-chip bias broadcast (saves 4MB DMA) ---
            # ones[1,128].T @ bias[1,512] on the (idle-at-startup) PE
            # replicates bias across partitions without touching the SDMA
            # engines the x-read fill is using.
            bias_c = cpool.tile([1, NB_C * DOUT], F32)
            nc.scalar.dma_start(bias_c[:], bca[:])
            ones = cpool.tile([1, 128], F32)
            nc.gpsimd.memset(ones[:], 1.0)
            bb = cpool.tile([128, NB_C // GRP, GRP, DOUT], F32)
            for g in range(NB_C // GRP):
                pb = popool.tile([CHUNK, GRP, DOUT], F32, tag="po")
                nc.tensor.matmul(
                    pb[:], ones[:], bias_c[:, g * GRP * DOUT:(g + 1) * GRP * DOUT],
                    start=True, stop=True,
                )
                nc.vector.tensor_copy(bb[:, g, :, :], pb[:])

            rep_cm = (
                tc.For_i(0, n_reps, 1) if n_reps > 1 else contextlib.nullcontext()
            )
            with rep_cm:
                main_body(nc, tc, xa, oa, w2, bb, ident,
                          xpool, xtpool, ptpool, popool, opool,
                          xh=xh, ob_sz=ob, plain_mm_transpose=plain_mm_transpose,
                          out_engine=out_engine, split_first=split_first)

    nc.compile()
    return nc


def main_body(nc, tc, xa, oa, w2, bb, ident,
              xpool, xtpool, ptpool, popool, opool,
              xh=XH, ob_sz=OB, plain_mm_transpose=False, out_engine="sync",
              split_first=0):
    wr = getattr(nc, out_engine)
    for c in range(NCHUNK):
        for h in range(NB_C // xh):
            x_t = xpool.tile([CHUNK, xh, DIN], F32, tag="x_t")
            if c == 0 and h == 0 and split_first:
                # Ramp-up: land the first blocks quickly so the PE starts
                # while the rest of the tile streams in.
                nc.sync.dma_start(
                    x_t[:, :split_first, :],
                    xa[0:CHUNK, 0:split_first, :],
                )
                nc.sync.dma_start(
                    x_t[:, split_first:, :],
                    xa[0:CHUNK, split_first:xh, :],
                )
            else:
                nc.sync.dma_start(
                    x_t[:],
                    xa[c * CHUNK:(c + 1) * CHUNK, h * xh:(h + 1) * xh, :],
                )
            for ob in range(xh // ob_sz):
                o_t = opool.tile([CHUNK, ob_sz, DOUT], F32)
                for gi in range(ob_sz // GRP):
                    blk0 = h * xh + ob * ob_sz + gi * GRP
                    g = blk0 // GRP
                    po = popool.tile([CHUNK, GRP, DOUT], F32)
                    for q in range(GRP // 2):
                        pair = blk0 // 2 + q
                        xoff = ob * ob_sz + gi * GRP + 2 * q
                        pt = ptpool.tile([128, CHUNK], F32)
                        if plain_mm_transpose:
                            nc.tensor.matmul(
                                pt[:], x_t[:, xoff:xoff + 2, :], ident[:],
                                start=True, stop=True,
                            )
                        else:
                            nc.tensor.transpose(
                                pt[:], x_t[:, xoff:xoff + 2, :], ident[:]
                            )
                        xts = xtpool.tile([128, CHUNK], F32)
                        nc.scalar.mul(xts[:], pt[:], 1.0)
                        nc.tensor.matmul(
                            po[:, 2 * q:2 * q + 2, :],
                            xts[:],
                            w2[:, pair, :],
                            start=True,
                            stop=True,
                        )
                    nc.vector.tensor_add(
                        o_t[:, gi * GRP:(gi + 1) * GRP, :],
                        po[:],
                        bb[:, g, :, :],
                    )
                nb0 = h * xh + ob * ob_sz
                wr.dma_start(
                    oa[c * CHUNK:(c + 1) * CHUNK, nb0:nb0 + ob_sz, :],
                    o_t[:],
                )


_PROGRAMS = {}


def get_program(n_reps=1):
    if n_reps not in _PROGRAMS:
        _PROGRAMS[n_reps] = build_program(n_reps)
    return _PROGRAMS[n_reps]


def prep_core_inputs(x, W, b, core):
    """Host-side shard + layout prep for one core."""
    n0, n1 = core * NB_C, (core + 1) * NB_C
    xs = np.ascontiguousarray(x[:, n0:n1, :], dtype=np.float32)
    Wk = W[n0:n1]                                  # [128, 64, 64] (n, o, i)
    WT = Wk.transpose(0, 2, 1)                     # [128, 64, 64] (n, i, o)
    # compact stacked layout [i2=128, pair, o]: rows 0:64 even blocks,
    # rows 64:128 odd blocks
    w2c = np.empty((128, NB_C // 2, DOUT), dtype=np.float32)
    w2c[:64] = WT[0::2].transpose(1, 0, 2)
    w2c[64:] = WT[1::2].transpose(1, 0, 2)
    bc = np.ascontiguousarray(b[n0:n1].reshape(1, NB_C * DOUT),
                              dtype=np.float32)
    ident = np.eye(128, dtype=np.float32)
    return {"x": xs, "w2c": w2c, "bc": bc, "ident": ident}


def make_in_maps(x, W, b):
    return [prep_core_inputs(x, W, b, k) for k in range(NCORES)]


def kernel(x, W, b):
    nc = get_program()
    in_maps = make_in_maps(x, W, b)
    res = run_bass_kernel_spmd(nc, in_maps, list(range(NCORES)))
    out = np.concatenate([res.results[k]["out"] for k in range(NCORES)], axis=1)
    return out


# revision 22
# speedup vs baseline: 1.0268x; 1.0040x over previous
"""Block-diagonal linear (BlockLinear) Trainium2 Bass kernel.

Problem: out[b, n, o] = sum_i x[b, n, i] * W[n, o, i] + bias[n, o]
  x: [1024, 1024, 64] f32, W: [1024, 64, 64] f32, bias: [1024, 64] f32

Sharding: block-parallel over n (num_blocks) across 8 NeuronCores;
each core owns 128 blocks. No inter-core communication.

Per-core algorithm (all fp32):
  - The contraction dim i is innermost in DRAM, so x tiles arrive in
    SBUF as [b=128 partitions, i free]. The tensor engine contracts over
    the partition dim, so x is transposed on chip: a PE transpose
    (x_tile.T @ I) over a [128b, 128] tile covering TWO blocks
    (2 x 64 = 128) yields xT [i2=128, b=128] in PSUM at full array width.
  - Weights are expanded on chip into block-pair block-diagonal tiles
    W2[pair] = [[W[2p].T, 0], [0, W[2p+1].T]]  (shape [128, 128]),
    so a single fp32 matmul  xT.T @ W2  = [b=128, o2=128] computes two
    blocks at once with K=128 (full partition utilization). Only the
    compact 2MB W.T is DMA'd; zeros + layout are built by DVE.
  - Bias is DMA'd compact (32KB), broadcast across partitions on chip by
    a PE ones-outer-product, and added by the DVE during the PSUM->SBUF
    copy of the output.
  - All DRAM<->SBUF DMAs move >=2KB contiguous per partition (line rate).
  - x reads ride the sync HWDGE ring; out writes + constants ride the
    scalar HWDGE ring so neither stream queues behind the other.

The kernel is memory-bound: per core it streams 32MB of x in and 32MB of
out at the measured ~300GB/s/core mixed R/W rate (~220us floor measured
for a pure-DMA loop with this access pattern); PE transposes/matmuls,
ACT copies, and DVE adds hide underneath (~231us measured end to end).
"""

import contextlib

import numpy as np

import concourse.bass as bass
import concourse.bacc as bacc
import concourse.tile as tile
from concourse import mybir
from concourse.bass_utils import run_bass_kernel_spmd

F32 = mybir.dt.float32

B = 1024          # batch
NB = 1024         # num_blocks (total)
DIN = 64
DOUT = 64
NCORES = 8
NB_C = NB // NCORES          # 128 blocks per core
CHUNK = 128                  # batch rows per tile (SBUF partitions)
NCHUNK = B // CHUNK          # 8
XH = 64                      # blocks per x DMA (16KB/partition)
OB = 32                      # blocks per out DMA (8KB/partition)
GRP = 8                      # blocks per PSUM bank group


def build_program(n_reps=1, xh=XH, ob=OB, pt_bufs=4, po_bufs=2,
                  xt_bufs=8, x_bufs=3, o_bufs=3, plain_mm_transpose=False,
                  out_engine="scalar", split_first=0):
    """n_reps>1 wraps the main loop in a HW loop repeating the whole
    computation — used only for timing (amortizes dispatch overhead)."""
    nc = bacc.Bacc(
        "TRN2", target_bir_lowering=False, debug=False, num_devices=NCORES
    )
    x_d = nc.dram_tensor("x", [B, NB_C, DIN], F32, kind="ExternalInput")
    # compact stacked W.T: rows 0:64 = W[2p].T, rows 64:128 = W[2p+1].T
    w2c_d = nc.dram_tensor("w2c", [128, NB_C // 2, DOUT], F32,
                           kind="ExternalInput")
    bc_d = nc.dram_tensor("bc", [1, NB_C * DOUT], F32, kind="ExternalInput")
    id_d = nc.dram_tensor("ident", [128, 128], F32, kind="ExternalInput")
    o_d = nc.dram_tensor("out", [B, NB_C, DOUT], F32, kind="ExternalOutput")

    xa, w2ca, bca, ida, oa = (t.ap() for t in (x_d, w2c_d, bc_d, id_d, o_d))

    with tile.TileContext(nc) as tc:
        with (
            tc.tile_pool(name="const", bufs=1) as cpool,
            tc.tile_pool(name="xin", bufs=x_bufs) as xpool,
            tc.tile_pool(name="xt", bufs=xt_bufs) as xtpool,
            tc.tile_pool(name="pt", bufs=pt_bufs, space="PSUM") as ptpool,
            tc.tile_pool(name="po", bufs=po_bufs, space="PSUM") as popool,
            tc.tile_pool(name="oo", bufs=o_bufs) as opool,
        ):
            ident = cpool.tile([128, 128], F32)
            nc.sync.dma_start(ident[:], ida[:])

            # Constants ride the scalar HWDGE ring so the sync ring's FIFO
            # leads with the first x tiles (compute starts sooner).
            # --- on-chip W2 block-diagonal expansion (saves 2MB DMA) ---
            w2 = cpool.tile([128, NB_C // 2, 128], F32)
            w2c = xpool.tile([128, NB_C // 2, DOUT], F32, tag="x_t")  # borrow slot
            nc.scalar.dma_start(w2c[:], w2ca[:])
            nc.gpsimd.memset(w2[:], 0.0)
            nc.vector.tensor_copy(w2[0:64, :, 0:64], w2c[0:64, :, :])
            nc.vector.tensor_copy(w2[64:128, :, 64:128], w2c[64:128, :, :])

            # --- on-chip bias broadcast (saves 4MB DMA) ---
            # ones[1,128].T @ bias[1,512] on the (idle-at-startup) PE
            # replicates bias across partitions without touching the SDMA
            # engines the x-read fill is using.
            bias_c = cpool.tile([1, NB_C * DOUT], F32)
            nc.scalar.dma_start(bias_c[:], bca[:])
            ones = cpool.tile([1, 128], F32)
            nc.gpsimd.memset(ones[:], 1.0)
            bb = cpool.tile([128, NB_C // GRP, GRP, DOUT], F32)
            for g in range(NB_C // GRP):
                pb = popool.tile([CHUNK, GRP, DOUT], F32, tag="po")
                nc.tensor.matmul(
                    pb[:], ones[:], bias_c[:, g * GRP * DOUT:(g + 1) * GRP * DOUT],
                    start=True, stop=True,
                )
                nc.vector.tensor_copy(bb[:, g, :, :], pb[:])

            rep_cm = (
                tc.For_i(0, n_reps, 1) if n_reps > 1 else contextlib.nullcontext()
            )
            with rep_cm:
                main_body(nc, tc, xa, oa, w2, bb, ident,
                          xpool, xtpool, ptpool, popool, opool,
                          xh=xh, ob_sz=ob, plain_mm_transpose=plain_mm_transpose,
                          out_engine=out_engine, split_first=split_first)

    nc.compile()
    return nc


def main_body(nc, tc, xa, oa, w2, bb, ident,
              xpool, xtpool, ptpool, popool, opool,
              xh=XH, ob_sz=OB, plain_mm_transpose=False, out_engine="sync",
              split_first=0):
    wr = getattr(nc, out_engine)
    for c in range(NCHUNK):
        for h in range(NB_C // xh):
            x_t = xpool.tile([CHUNK, xh, DIN], F32, tag="x_t")
            if c == 0 and h == 0 and split_first:
                # Ramp-up: land the first blocks quickly so the PE starts
                # while the rest of the tile streams in.
                nc.sync.dma_start(
                    x_t[:, :split_first, :],
                    xa[0:CHUNK, 0:split_first, :],
                )
                nc.sync.dma_start(
                    x_t[:, split_first:, :],
                    xa[0:CHUNK, split_first:xh, :],
                )
            else:
                nc.sync.dma_start(
                    x_t[:],
                    xa[c * CHUNK:(c + 1) * CHUNK, h * xh:(h + 1) * xh, :],
                )
            for ob in range(xh // ob_sz):
                o_t = opool.tile([CHUNK, ob_sz, DOUT], F32)
                for gi in range(ob_sz // GRP):
                    blk0 = h * xh + ob * ob_sz + gi * GRP
                    g = blk0 // GRP
                    po = popool.tile([CHUNK, GRP, DOUT], F32)
                    for q in range(GRP // 2):
                        pair = blk0 // 2 + q
                        xoff = ob * ob_sz + gi * GRP + 2 * q
                        pt = ptpool.tile([128, CHUNK], F32)
                        if plain_mm_transpose:
                            nc.tensor.matmul(
                                pt[:], x_t[:, xoff:xoff + 2, :], ident[:],
                                start=True, stop=True,
                            )
                        else:
                            nc.tensor.transpose(
                                pt[:], x_t[:, xoff:xoff + 2, :], ident[:]
                            )
                        xts = xtpool.tile([128, CHUNK], F32)
                        nc.scalar.mul(xts[:], pt[:], 1.0)
                        nc.tensor.matmul(
                            po[:, 2 * q:2 * q + 2, :],
                            xts[:],
                            w2[:, pair, :],
                            start=True,
                            stop=True,
                        )
                    nc.vector.tensor_add(
                        o_t[:, gi * GRP:(gi + 1) * GRP, :],
                        po[:],
                        bb[:, g, :, :],
                    )
                nb0 = h * xh + ob * ob_sz
                wr.dma_start(
                    oa[c * CHUNK:(c + 1) * CHUNK, nb0:nb0 + ob_sz, :],
                    o_t[:],
                )


_PROGRAMS = {}


def get_program(n_reps=1):
    if n_reps not in _PROGRAMS:
        _PROGRAMS[n_reps] = build_program(n_reps)
    return _PROGRAMS[n_reps]


def prep_core_inputs(x, W, b, core):
    """Host-side shard + layout prep for one core."""
    n0, n1 = core * NB_C, (core + 1) * NB_C
    xs = np.ascontiguousarray(x[:, n0:n1, :], dtype=np.float32)
    Wk = W[n0:n1]                                  # [128, 64, 64] (n, o, i)
    WT = Wk.transpose(0, 2, 1)                     # [128, 64, 64] (n, i, o)
    # compact stacked layout [i2=128, pair, o]: rows 0:64 even blocks,
    # rows 64:128 odd blocks
    w2c = np.empty((128, NB_C // 2, DOUT), dtype=np.float32)
    w2c[:64] = WT[0::2].transpose(1, 0, 2)
    w2c[64:] = WT[1::2].transpose(1, 0, 2)
    bc = np.ascontiguousarray(b[n0:n1].reshape(1, NB_C * DOUT),
                              dtype=np.float32)
    ident = np.eye(128, dtype=np.float32)
    return {"x": xs, "w2c": w2c, "bc": bc, "ident": ident}


def make_in_maps(x, W, b):
    return [prep_core_inputs(x, W, b, k) for k in range(NCORES)]


def kernel(x, W, b):
    nc = get_program()
    in_maps = make_in_maps(x, W, b)
    res = run_bass_kernel_spmd(nc, in_maps, list(range(NCORES)))
    out = np.concatenate([res.results[k]["out"] for k in range(NCORES)], axis=1)
    return out
